# revision 12
# baseline (speedup 1.0000x reference)
"""Trainium2 Bass kernel for ConvMosaic: 3x3 conv (pad 1) where the weight set
depends on output position p%16 == w%16 (column phase).

Strategy (8 NeuronCores, SPMD):
  - Shard over (N, H): core k handles image k//2, row-half k%2 (128 rows).
  - Host pre-builds a phase-deinterleaved bf16 image per core, partitioned
    by ROW GROUP (no dj replication -- input is only ~2.5MB/core):
      x_sb[(g*32+c), o, hh, wg] = xpad[c, g*32+hh, 16*wg + o]
    for o in 0..17, hh in 0..33 (32-row group + 2 halo rows).
  - The 128x128 PE array is row-tiled into four 32-row strips (strip = row
    group g) x two 64-col halves (phases 2i, 2i+1): per tap t=(di,dj) of
    phase s, strip g runs a K=32 matmul with tile_position (32g, 0|64),
    moving slice x_sb[32g:32g+32, s+dj, di:di+32, :] (contiguous 512 elems),
    accumulating 9 taps into PSUM bank g [128=(half,oc), 32h, 16wg].
    8 concurrent matmuls per tap slot -> full PE column utilization; each
    strip's LDWEIGHTS overlaps other strips' matmuls (disjoint row groups).
  - Weights are host-replicated per strip: w_sb[(g*32+c), s, t, oc],
    DMAed in per-pair chunks on the scalar ring (parallel with input).
  - Pair-outer pipeline: pair i needs input planes 2i..2i+3 only; computes
    all 4 strips, evacuates banks with alternating DVE/ACT f32->bf16 copies
    into per-pair [128, ROWS, WG] staging, DMAs out immediately (2-pair
    blocks for bigger packets).
  - Host reassembles w = 16*wg + 2*i + half and upcasts to f32.
"""

import sys

import numpy as np

for _p in ("/opt/trn_rl_repo",):
    if _p not in sys.path:
        sys.path.insert(0, _p)

N, C, H, W = 4, 32, 256, 256
OC = 64
SPE = 16
NCORES = 8
ROWS = H * N // NCORES  # 128 rows per core
GR = 32  # rows per group/strip
GROUPS = ROWS // GR
NPAIR = SPE // 2  # 8 phase pairs (2i, 2i+1)
WG = W // 16  # 16 column groups
NPL = 18  # deinterleave planes o = s + dj
HH = GR + 2  # 34 rows per strip incl halo

_cache = {}


def build_nc():
    from concourse import bacc, bass, mybir, tile

    f32 = mybir.dt.float32
    bf16 = mybir.dt.bfloat16

    nc = bacc.Bacc()
    xin = nc.dram_tensor("xin", [4 * C, NPL, HH, WG], bf16, kind="ExternalInput")
    wdr = nc.dram_tensor("w", [4 * C, SPE, 9, OC], bf16, kind="ExternalInput")
    # out partition dim = (half, oc); free = (pair, row, wg); w = 16*wg+2*pair+half
    out = nc.dram_tensor("out", [2 * OC, NPAIR, ROWS, WG], bf16, kind="ExternalOutput")

    with tile.TileContext(nc) as tc:
        with (
            tc.tile_pool(name="wp", bufs=1) as wp,
            tc.tile_pool(name="xp", bufs=1) as xp,
            tc.tile_pool(name="op", bufs=3) as op,
            tc.tile_pool(name="pp", bufs=8, space=bass.MemorySpace.PSUM) as pp,
        ):
            # weights share the sync ring with x (scalar ring = output only);
            # first chunks are small so pair 0's matmuls start ASAP
            w_sb = wp.tile([4 * C, SPE, 9, OC], bf16)
            x_sb = xp.tile([4 * C, NPL, HH, WG], bf16)
            nc.sync.dma_start(w_sb[:, 0:2], wdr[:, 0:2])
            nc.sync.dma_start(x_sb[:, 0:4], xin[:, 0:4])
            nc.sync.dma_start(w_sb[:, 2:8], wdr[:, 2:8])
            nc.sync.dma_start(x_sb[:, 4:10], xin[:, 4:10])
            nc.sync.dma_start(w_sb[:, 8:16], wdr[:, 8:16])
            nc.sync.dma_start(x_sb[:, 10:18], xin[:, 10:18])

            for ii in range(NPAIR // 2):
                o_sb = op.tile([2 * OC, 2, ROWS, WG], bf16)
                for b in range(2):
                    i = 2 * ii + b
                    sA, sB = 2 * i, 2 * i + 1
                    psl = [
                        pp.tile([2 * OC, GR, WG], f32, tag="ps", name=f"ps_{i}_{g}")
                        for g in range(GROUPS)
                    ]
                    for t in range(9):
                        di, dj = t // 3, t % 3
                        for g in range(GROUPS):
                            nc.tensor.matmul(
                                psl[g][0:OC],
                                w_sb[g * C : (g + 1) * C, sA, t, :],
                                x_sb[g * C : (g + 1) * C, sA + dj, di : di + GR, :],
                                start=(t == 0),
                                stop=(t == 8),
                                tile_position=(g * C, 0),
                            )
                            nc.tensor.matmul(
                                psl[g][OC : 2 * OC],
                                w_sb[g * C : (g + 1) * C, sB, t, :],
                                x_sb[g * C : (g + 1) * C, sB + dj, di : di + GR, :],
                                start=(t == 0),
                                stop=(t == 8),
                                tile_position=(g * C, OC),
                            )
                    for g in range(GROUPS):
                        dst = o_sb[:, b, g * GR : (g + 1) * GR, :]
                        if g % 2 == 0:
                            nc.vector.tensor_copy(dst, psl[g][:])
                        else:
                            nc.scalar.activation(
                                dst, psl[g][:], mybir.ActivationFunctionType.Copy
                            )
                nc.scalar.dma_start(out[:, 2 * ii : 2 * ii + 2, :, :], o_sb[:])
    nc.finalize()
    return nc


def shard_inputs(x, weight):
    import ml_dtypes

    bf = np.dtype(ml_dtypes.bfloat16)
    x = np.asarray(x, dtype=np.float32)
    weight = np.asarray(weight, dtype=np.float32)
    xpad = np.zeros((N, C, H + 2, W + 2), np.float32)
    xpad[:, :, 1:-1, 1:-1] = x
    # xv[n, c, h', wg, o] = xpad[n, c, h', 16*wg + o], o in 0..17
    xv = np.lib.stride_tricks.sliding_window_view(xpad, NPL, axis=3)[:, :, :, ::16, :]
    xv = xv.astype(bf)
    # wh[(g*32+c), s, t, oc] = weight[s, c*9+t, oc], replicated over g
    wh1 = np.ascontiguousarray(
        weight.reshape(SPE, C, 9, OC).transpose(1, 0, 2, 3)
    ).astype(bf)  # [C, SPE, 9, OC]
    wh = np.ascontiguousarray(np.broadcast_to(wh1, (4, C, SPE, 9, OC))).reshape(
        4 * C, SPE, 9, OC
    )
    in_maps = []
    for k in range(NCORES):
        n, r0 = k // 2, (k % 2) * ROWS
        xc = np.empty((GROUPS, C, NPL, HH, WG), bf)
        for g in range(GROUPS):
            blk = xv[n, :, r0 + g * GR : r0 + g * GR + HH, :, :]  # [C, HH, WG, NPL]
            xc[g] = blk.transpose(0, 3, 1, 2)
        in_maps.append({"xin": xc.reshape(4 * C, NPL, HH, WG), "w": wh})
    return in_maps


def unshard_outputs(results):
    out = np.empty((N, OC, H, W), np.float32)
    for k in range(NCORES):
        n, r0 = k // 2, (k % 2) * ROWS
        od = np.asarray(results[k]["out"]).astype(np.float32)
        od = od.reshape(2, OC, NPAIR, ROWS, WG)
        # w = 16*wg + 2*pair + half  ->  order (oc, row, wg, pair, half)
        out[n, :, r0 : r0 + ROWS, :] = od.transpose(1, 3, 4, 2, 0).reshape(
            OC, ROWS, W
        )
    return out


def run(x, weight, **spmd_kwargs):
    from concourse.bass_utils import run_bass_kernel_spmd

    in_maps = shard_inputs(x, weight)
    if "nc" not in _cache:
        _cache["nc"] = build_nc()
    res = run_bass_kernel_spmd(_cache["nc"], in_maps, list(range(NCORES)), **spmd_kwargs)
    return unshard_outputs(res.results), res


def kernel(x, weight):
    out, _ = run(x, weight)
    return out
